# revision 42
# baseline (speedup 1.0000x reference)
"""Bilateral effect kernel for Trainium2 (8 NeuronCores, SPMD).

Algorithm (matches reference.py):
  For each pixel p and tap delta=(j,i), j in [-4,4], i in [1,4] (taps with
  max(i,|j|)=5 are never active since sigmaD<4):
    w(p,+d) = exp(-(E(p,p+d)*v(p) + A_k(p)))
    w(p,-d) = exp(-(E(p-d,p)*v(p) + A_k(p)))
    A_k     = d2*u + 100*(tap k inactive),  u = 1/(0.5*sigmaD^2+eps)
    E(a,b)  = sum_c scale_c^2 (x_c[a]-x_c[b])^2,  scale=(100,254,254)
    v = 1/(2*sigmaR^2+eps)
    out_c = (x_c + sum w*x_c[shifted]) / (1 + sum w)
  (exp(-100) underflows to exactly 0 in fp16 -> the inactive-tap mask folded
   into the host-precomputed A planes is exact.)

Layout: every NeuronCore gets 64 image rows = 128 sub-tiles of 16x16 center
pixels; each SBUF partition owns one sub-tile padded to 24x24, stored as 4
fp16 planes [x0,x1,x2,ones] (halo+edge replication+scaling done host-side).
All taps are pure free-dim shifted reads.

Engine split (taps processed as (j,i)/(-j,i) pairs):
  DVE : per tap a planar 3-ch sub, E = channel-sum (2 pair-merged adds),
        E*v (2 dirs packed per op), +A add, 2 broadcast prod mults.
  ACT : one Square per pair (both taps' diffs), one exp per pair writing
        all 4 dirs' w straight into the combined prod/w buffer.
  PE  : psum += I @ [prod3|w] (512-col fp16 matmuls) -- numerator AND
        denominator accumulate on the tensor engine; A = d2*u + mask and
        v = 16/(2*sigmaR^2+eps) come precomputed from the host.
"""
import dataclasses
import numpy as np

import concourse.bass as bass
import concourse.mybir as mybir
import concourse.tile as tile
from concourse.bass_utils import run_bass_kernel_spmd
from concourse.masks import make_identity

F32 = mybir.dt.float32
F16 = mybir.dt.float16
ALU = mybir.AluOpType
ACTF = mybir.ActivationFunctionType

H = W = 512
NCORES = 8
T = 16            # center tile side
PAD = 4           # halo
PT = T + 2 * PAD  # 24 padded tile side
NP = 128          # partitions (tiles) per core
TRC = 32          # tile-cols per core (512/16); tile-rows per core = 4
EPS = float(np.finfo(np.float32).eps)
SCALE = (100.0, 254.0, 254.0)
NPIX = T * T      # 256
PP = PT * PT      # 576 plane size
MAXNEL = 20 * 20  # max extended-window size

# live taps: (j=row off, i=col off, d2, m), ordered so that each group is
# processed together (one ACT square / one exp per group). (j,i)/(-j,i) are
# natural pairs; the j=0 taps are pseudo-paired (padded to the wider window).
_PAIR_KEYS = [[(j, i), (-j, i)] for i in range(1, 5) for j in range(1, 5)]
_PAIR_KEYS += [[(0, 1), (0, 2)], [(0, 3), (0, 4)]]
TAPS = [(j, i, float(i * i + j * j), max(i, abs(j)))
        for grp in _PAIR_KEYS for (j, i) in grp]
assert len(TAPS) == 36
_k = iter(range(36))
PAIRS = [[next(_k) for _ in grp] for grp in _PAIR_KEYS]


def _sub(ap, dims, off):
    """AP over free dims of a pool tile: dims = [[step,count],...] (elements),
    off = element offset within the partition's free space."""
    return dataclasses.replace(
        ap, ap=[list(ap.ap[0])] + [[int(s), int(c)] for s, c in dims],
        offset=int(off))


def _patch_sem_clear():
    """The walrus build in this container rejects the
    EVENT_SEMAPHORE_RANGE_CLEAR InstISA that Tile's kernel-tail drain emits
    ("ISA wrong length").  Replace it with per-semaphore nops carrying
    sem-wr-imm(0) updates, keeping the original free-list bookkeeping."""
    if getattr(bass.Bass, "_semclear_patched", False):
        return
    from concourse.bass import SemaphoreHandle

    def clear_and_free_semaphores(self, sems):
        if not sems:
            return
        sem_nums = [s.num if isinstance(s, SemaphoreHandle) else s for s in sems]
        self.gpsimd.dma_reset(range(min(sem_nums), max(sem_nums) + 1))
        for n in sem_nums:
            inst = self.gpsimd.nop()
            inst.sync_info = mybir.SyncInfo(
                on_wait=[],
                on_update=[mybir.SyncUpdate(
                    sync_type="semaphore", id=int(n),
                    update_mode="sem-wr-imm", update_value=0)])
        self._state.prepend_free_semaphores(sem_nums)
        for poison_set in self._tile_sem_poison_stack:
            poison_set.update(sem_nums)

    bass.Bass.clear_and_free_semaphores = clear_and_free_semaphores
    bass.Bass._semclear_patched = True


# These either never carry inline waits or are sequencer-level (multi-wait ok).
_WAIT_EXEMPT = {
    "InstDMA", "InstDMACopy", "InstDmaTransposeAnt", "InstTensorLoad",
    "InstTensorSave", "InstEventSemaphore",
    "InstCall", "InstUnconditionalBranch", "InstISA", "InstRegisterMove",
}


def _legalize_waits(nc):
    """This container's walrus accepts at most ONE inline sync wait per
    compute instruction.  Split extras onto same-engine NoOps inserted just
    before the instruction (engine stalls at the nop first — semantics
    preserved)."""
    cnt = 0
    for f in nc.m.functions:
        for blk in f.blocks:
            out = []
            for inst in blk.instructions:
                si = inst.sync_info
                if (si is not None and len(si.on_wait) > 1
                        and type(inst).__name__ not in _WAIT_EXEMPT):
                    waits = list(si.on_wait)
                    for wextra in waits[:-1]:
                        nop = mybir.InstNoOp(
                            name=f"waitnop-{cnt}", engine=inst.engine)
                        cnt += 1
                        nop.sync_info = mybir.SyncInfo(
                            on_wait=[wextra], on_update=[])
                        out.append(nop)
                    inst.sync_info = mybir.SyncInfo(
                        on_wait=[waits[-1]], on_update=list(si.on_update))
                out.append(inst)
            blk.instructions = out
    return cnt


def build_program():
    _patch_sem_clear()
    nc = bass.Bass("TRN2")
    xin = nc.dram_tensor("xin", [NP, 4 * PP], F16, kind="ExternalInput")
    vin = nc.dram_tensor("vin", [NP, NPIX], F16, kind="ExternalInput")
    ain = nc.dram_tensor("ain", [NP, 36 * NPIX], F16, kind="ExternalInput")
    oout = nc.dram_tensor("oout", [NP, 4 * NPIX], F32, kind="ExternalOutput")

    with tile.TileContext(nc) as tc, \
         nc.allow_low_precision(reason="fp16 main path; fp32 psum accum"):
        with tc.tile_pool(name="persist", bufs=1) as pp, \
             tc.tile_pool(name="work", bufs=6) as wp, \
             tc.tile_pool(name="psum", bufs=1, space="PSUM") as qp:
            X = pp.tile([NP, 4 * PP], F16, tag="X")
            v = pp.tile([NP, NPIX], F16, tag="v")
            A = pp.tile([NP, 36 * NPIX], F16, tag="A")
            ident = pp.tile([128, 128], F16, tag="ident")
            ob = pp.tile([NP, 4 * NPIX], F32, tag="ob")

            # split X across both HWDGE queues to halve the startup stall
            nc.sync.dma_start(X[:, 0:2 * PP], xin[:, 0:2 * PP])
            nc.scalar.dma_start(X[:, 2 * PP:4 * PP], xin[:, 2 * PP:4 * PP])
            nc.sync.dma_start(v[:, :], vin[:, :])
            nc.sync.dma_start(A[:, :], ain[:, :])
            make_identity(nc, ident[:, :])

            psumA = qp.tile([128, 512], F32, tag="psA")  # planes x0,x1
            psumB = qp.tile([128, 512], F32, tag="psB")  # planes x2,den

            xap = X[:, :]
            CENTER = PAD * PT + PAD

            # center term: psum <- [x0,x1] , [x2,1] (weight exactly 1).
            # NOTE: start=True resets the whole PSUM bank -- exactly one
            # start per psum tile.
            cA = _sub(xap, [[PP, 2], [PT, T], [1, T]], CENTER)
            cB = _sub(xap, [[PP, 2], [PT, T], [1, T]], 2 * PP + CENTER)
            nc.tensor.matmul(psumA[:, :], ident[:, :], cA,
                             start=True, stop=False)
            mm = nc.tensor.matmul(psumB[:, :], ident[:, :], cB,
                                  start=True, stop=False)
            mm.ldweights = False  # identity stays resident in the PE array

            for pi, pair in enumerate(PAIRS):
                npr = len(pair)
                last_pair = pi == len(PAIRS) - 1
                # D = [rawA3 | rawB3 | sqA3 | sqB3] planes at stride MAXNEL
                D = wp.tile([NP, 12 * MAXNEL], F16, tag="D")
                E = wp.tile([NP, 2 * MAXNEL], F16, tag="E")
                t2p = wp.tile([NP, 1024], F16, tag="t2")
                s2p = wp.tile([NP, 1024], F16, tag="s2")
                C = wp.tile([NP, 4096], F16, tag="C")

                # pseudo-pairs (unequal i) pad to the wider window; the
                # extra columns read in-bounds halo junk that is never used
                j0 = TAPS[pair[0]][0]
                nr = T + abs(j0)
                ncol = T + max(TAPS[t][1] for t in pair)
                nel = nr * ncol

                for s, ti in enumerate(pair):
                    j, i, d2, m = TAPS[ti]
                    rlo_s = min(0, -j)
                    w0 = (PAD + rlo_s) * PT + (PAD - i)
                    w1 = (PAD + rlo_s + j) * PT + PAD
                    in0 = _sub(xap, [[PP, 3], [PT, nr], [1, ncol]], w0)
                    in1 = _sub(xap, [[PP, 3], [PT, nr], [1, ncol]], w1)
                    dap = _sub(D[:, :], [[MAXNEL, 3], [1, nel]],
                               s * 3 * MAXNEL)
                    nc.vector.tensor_tensor(out=dap, in0=in0, in1=in1,
                                            op=ALU.subtract)

                # one ACT square for the whole pair
                dln = _sub(D[:, :], [[1, npr * 3 * MAXNEL]], 0)
                dsq = _sub(D[:, :], [[1, npr * 3 * MAXNEL]], 6 * MAXNEL)
                nc.scalar.activation(dsq, dln, ACTF.Square)

                # E_s = sq0 + sq1 + sq2 (both taps in one op)
                e = _sub(E[:, :], [[MAXNEL, npr], [1, nel]], 0)
                nc.vector.tensor_tensor(
                    out=e,
                    in0=_sub(D[:, :], [[3 * MAXNEL, npr], [1, nel]],
                             6 * MAXNEL),
                    in1=_sub(D[:, :], [[3 * MAXNEL, npr], [1, nel]],
                             7 * MAXNEL), op=ALU.add)
                nc.vector.tensor_tensor(
                    out=e, in0=e,
                    in1=_sub(D[:, :], [[3 * MAXNEL, npr], [1, nel]],
                             8 * MAXNEL), op=ALU.add)

                slot_dirs = []
                for s, ti in enumerate(pair):
                    j, i, d2, m = TAPS[ti]
                    rlo_s = min(0, -j)
                    # E(q) = ||x(q) - x(q+d)||^2 for q in the extended window
                    offd = ((0 - rlo_s) * ncol + i,
                            (-j - rlo_s) * ncol + 0)
                    base = min(offd) + s * MAXNEL
                    step = abs(offd[1] - offd[0])
                    slot_dir = (0, 1) if offd[0] <= offd[1] else (1, 0)
                    slot_dirs.append(slot_dir)

                    e2 = _sub(E[:, :], [[step, 2], [ncol, T], [1, T]], base)
                    v2 = _sub(v[:, :], [[0, 2], [1, NPIX]], 0)
                    t2a = _sub(t2p[:, :], [[1, 512]], s * 512)
                    nc.vector.tensor_tensor(out=t2a, in0=e2, in1=v2,
                                            op=ALU.mult)

                # s2 = t2 + A, both taps in one op (taps adjacent in A)
                nc.vector.tensor_tensor(
                    out=_sub(s2p[:, :], [[1, npr * 512]], 0),
                    in0=_sub(t2p[:, :], [[1, npr * 512]], 0),
                    in1=_sub(A[:, :], [[NPIX, npr], [0, 2], [1, NPIX]],
                             pair[0] * NPIX),
                    op=ALU.add)

                # one exp for the whole pair -> C's w slots
                sin = _sub(s2p[:, :], [[1, npr * 512]], 0)
                wap = _sub(C[:, :], [[1024, 2 * npr], [1, NPIX]], 768)
                nc.scalar.activation(wap, sin, ACTF.Exp, scale=-1.0)

                # prod3 = w * [x0,x1,x2](shifted) per tap-slot and direction
                for s, ti in enumerate(pair):
                    j, i, d2, m = TAPS[ti]
                    for slot in range(2):
                        d = slot_dirs[s][slot]
                        sgn = 1 if d == 0 else -1
                        co = s * 2048 + slot * 1024
                        wbr = _sub(C[:, :], [[0, 3], [1, NPIX]], co + 768)
                        xw = _sub(xap, [[PP, 3], [PT, T], [1, T]],
                                  (PAD + sgn * j) * PT + (PAD + sgn * i))
                        pr = _sub(C[:, :], [[1, 768]], co)
                        nc.vector.tensor_tensor(out=pr, in0=wbr, in1=xw,
                                                op=ALU.mult)

                # psum accumulation on PE: A += [p0,p1], B += [p2,w]
                for s in range(npr):
                    for slot in range(2):
                        stop = last_pair and s == npr - 1 and slot == 1
                        co = s * 2048 + slot * 1024
                        rA = _sub(C[:, :], [[1, 512]], co)
                        rB = _sub(C[:, :], [[1, 512]], co + 512)
                        mm = nc.tensor.matmul(psumA[:, :], ident[:, :], rA,
                                              start=False, stop=stop)
                        mm.ldweights = False
                        mm = nc.tensor.matmul(psumB[:, :], ident[:, :], rB,
                                              start=False, stop=stop)
                        mm.ldweights = False

            # ship raw [num0,num1,num2,den] planes; host divides + rescales
            # (ACT engine is idle at the tail; PSUM cannot DMA directly)
            nc.scalar.activation(_sub(ob[:, :], [[1, 512]], 0),
                                 psumA[:, :], ACTF.Copy)
            nc.scalar.activation(_sub(ob[:, :], [[1, 512]], 512),
                                 psumB[:, :], ACTF.Copy)
            nc.sync.dma_start(oout[:, :], ob[:, :])
    _legalize_waits(nc)
    return nc


def host_shard(x, sigmaD, sigmaR):
    """x [1,3,512,512] -> per-core inputs. Pure gather/pad/scale prep."""
    from numpy.lib.stride_tricks import sliding_window_view
    xs = x[0] * (np.array(SCALE, np.float32) / 4.0)[:, None, None]
    xg = np.pad(xs, ((0, 0), (PAD, PAD), (PAD, PAD)), mode="edge")
    swv = sliding_window_view(xg, (PT, PT), axis=(1, 2))
    blocks = swv[:, ::T, ::T][:, :32, :32]                # [3, 32, 32, 24, 24]
    ones = np.ones((1,) + blocks.shape[1:], np.float32)
    x4 = np.concatenate([blocks, ones], axis=0)           # [4, 32, 32, 24, 24]
    tiles = np.ascontiguousarray(
        x4.transpose(1, 2, 0, 3, 4)).astype(np.float16)   # [32,32,4,24,24]
    tiles = tiles.reshape(NCORES, NP, 4 * PP)

    sd, sr = sigmaD[0, 0], sigmaR[0, 0]
    u = 1.0 / (0.5 * sd * sd + EPS)
    v16 = 16.0 / (2.0 * sr * sr + EPS)
    # A_k = d2_k*u + 100*(tap k inactive)
    A = np.stack([d2 * u + 100.0 * (sd <= float(m - 1))
                  for (_, _, d2, m) in TAPS])

    def tile_sig(s):  # [k?,512,512] -> [NCORES, NP, k?*256] tile-major
        k = s.shape[0] if s.ndim == 3 else 1
        s = s.reshape(k, 32, T, 32, T).transpose(1, 3, 0, 2, 4)
        return np.ascontiguousarray(s).reshape(NCORES, NP, k * NPIX)

    vt = tile_sig(v16[None]).astype(np.float16)
    at = tile_sig(A).astype(np.float16)
    return [{"xin": tiles[c], "vin": vt[c], "ain": at[c]}
            for c in range(NCORES)]


def assemble(results):
    rescale = (4.0 / np.array(SCALE, np.float32))[:, None, None]
    out = np.empty((1, 3, H, W), np.float32)
    for c, r in enumerate(results):
        o = r["oout"].reshape(4, TRC, 4, T, T)
        # [tr, tc, plane, r, cc] -> [plane, tr, r, tc, cc]
        o = o.transpose(2, 0, 3, 1, 4).reshape(4, 64, W)
        out[0, :, c * 64:(c + 1) * 64, :] = o[:3] * rescale / o[3]
    return out


_NC_CACHE = {}


def get_nc():
    if "nc" not in _NC_CACHE:
        _NC_CACHE["nc"] = build_program()
    return _NC_CACHE["nc"]


def kernel(x, sigmaD, sigmaR, trace=False):
    x = np.asarray(x, np.float32)
    sigmaD = np.asarray(sigmaD, np.float32)
    sigmaR = np.asarray(sigmaR, np.float32)
    in_maps = host_shard(x, sigmaD, sigmaR)
    nc = get_nc()
    res = run_bass_kernel_spmd(nc, in_maps, list(range(NCORES)), trace=trace)
    out = assemble(res.results)
    kernel.last_result = res
    return out


# revision 46
# speedup vs baseline: 1.0058x; 1.0058x over previous
"""Bilateral effect kernel for Trainium2 (8 NeuronCores, SPMD).

Algorithm (matches reference.py):
  For each pixel p and tap delta=(j,i), j in [-4,4], i in [1,4] (taps with
  max(i,|j|)=5 are never active since sigmaD<4):
    w(p,+d) = exp(-(E(p,p+d)*v(p) + A_k(p)))
    w(p,-d) = exp(-(E(p-d,p)*v(p) + A_k(p)))
    A_k     = d2*u + 100*(tap k inactive),  u = 1/(0.5*sigmaD^2+eps)
    E(a,b)  = sum_c scale_c^2 (x_c[a]-x_c[b])^2,  scale=(100,254,254)
    v = 1/(2*sigmaR^2+eps)
    out_c = (x_c + sum w*x_c[shifted]) / (1 + sum w)
  (exp(-100) underflows to exactly 0 in fp16 -> the inactive-tap mask folded
   into the host-precomputed A planes is exact.)

Layout: every NeuronCore gets 64 image rows = 128 sub-tiles of 16x16 center
pixels; each SBUF partition owns one sub-tile padded to 24x24, stored as 4
fp16 planes [x0,x1,x2,ones] (halo+edge replication+scaling done host-side).
All taps are pure free-dim shifted reads.

Engine split (taps processed as (j,i)/(-j,i) pairs):
  DVE : per tap a planar 3-ch sub, E = channel-sum (2 pair-merged adds),
        E*v (2 dirs packed per op), +A add, 2 broadcast prod mults.
  ACT : one Square per pair (both taps' diffs), one exp per pair writing
        all 4 dirs' w straight into the combined prod/w buffer.
  PE  : psum += I @ [prod3|w] (512-col fp16 matmuls) -- numerator AND
        denominator accumulate on the tensor engine; A = d2*u + mask and
        v = 16/(2*sigmaR^2+eps) come precomputed from the host.
"""
import dataclasses
import numpy as np

import concourse.bass as bass
import concourse.mybir as mybir
import concourse.tile as tile
from concourse.bass_utils import run_bass_kernel_spmd
from concourse.masks import make_identity

F32 = mybir.dt.float32
F16 = mybir.dt.float16
ALU = mybir.AluOpType
ACTF = mybir.ActivationFunctionType

H = W = 512
NCORES = 8
T = 16            # center tile side
PAD = 4           # halo
PT = T + 2 * PAD  # 24 padded tile side
NP = 128          # partitions (tiles) per core
TRC = 32          # tile-cols per core (512/16); tile-rows per core = 4
EPS = float(np.finfo(np.float32).eps)
SCALE = (100.0, 254.0, 254.0)
NPIX = T * T      # 256
PP = PT * PT      # 576 plane size
MAXNEL = 20 * 20  # max extended-window size

# live taps: (j=row off, i=col off, d2, m), ordered so that each group is
# processed together (one ACT square / one exp per group). (j,i)/(-j,i) are
# natural pairs; the j=0 taps are pseudo-paired (padded to the wider window).
_PAIR_KEYS = [[(j, i), (-j, i)] for i in range(1, 5) for j in range(1, 5)]
_PAIR_KEYS += [[(0, 1), (0, 2)], [(0, 3), (0, 4)]]
TAPS = [(j, i, float(i * i + j * j), max(i, abs(j)))
        for grp in _PAIR_KEYS for (j, i) in grp]
assert len(TAPS) == 36
_k = iter(range(36))
PAIRS = [[next(_k) for _ in grp] for grp in _PAIR_KEYS]


def _sub(ap, dims, off):
    """AP over free dims of a pool tile: dims = [[step,count],...] (elements),
    off = element offset within the partition's free space."""
    return dataclasses.replace(
        ap, ap=[list(ap.ap[0])] + [[int(s), int(c)] for s, c in dims],
        offset=int(off))


def _patch_sem_clear():
    """The walrus build in this container rejects the
    EVENT_SEMAPHORE_RANGE_CLEAR InstISA that Tile's kernel-tail drain emits
    ("ISA wrong length").  Replace it with per-semaphore nops carrying
    sem-wr-imm(0) updates, keeping the original free-list bookkeeping."""
    if getattr(bass.Bass, "_semclear_patched", False):
        return
    from concourse.bass import SemaphoreHandle

    def clear_and_free_semaphores(self, sems):
        if not sems:
            return
        sem_nums = [s.num if isinstance(s, SemaphoreHandle) else s for s in sems]
        self.gpsimd.dma_reset(range(min(sem_nums), max(sem_nums) + 1))
        for n in sem_nums:
            inst = self.gpsimd.nop()
            inst.sync_info = mybir.SyncInfo(
                on_wait=[],
                on_update=[mybir.SyncUpdate(
                    sync_type="semaphore", id=int(n),
                    update_mode="sem-wr-imm", update_value=0)])
        self._state.prepend_free_semaphores(sem_nums)
        for poison_set in self._tile_sem_poison_stack:
            poison_set.update(sem_nums)

    bass.Bass.clear_and_free_semaphores = clear_and_free_semaphores
    bass.Bass._semclear_patched = True


# These either never carry inline waits or are sequencer-level (multi-wait ok).
_WAIT_EXEMPT = {
    "InstDMA", "InstDMACopy", "InstDmaTransposeAnt", "InstTensorLoad",
    "InstTensorSave", "InstEventSemaphore",
    "InstCall", "InstUnconditionalBranch", "InstISA", "InstRegisterMove",
}


def _legalize_waits(nc):
    """This container's walrus accepts at most ONE inline sync wait per
    compute instruction.  Split extras onto same-engine NoOps inserted just
    before the instruction (engine stalls at the nop first — semantics
    preserved)."""
    cnt = 0
    for f in nc.m.functions:
        for blk in f.blocks:
            out = []
            for inst in blk.instructions:
                si = inst.sync_info
                if (si is not None and len(si.on_wait) > 1
                        and type(inst).__name__ not in _WAIT_EXEMPT):
                    waits = list(si.on_wait)
                    for wextra in waits[:-1]:
                        nop = mybir.InstNoOp(
                            name=f"waitnop-{cnt}", engine=inst.engine)
                        cnt += 1
                        nop.sync_info = mybir.SyncInfo(
                            on_wait=[wextra], on_update=[])
                        out.append(nop)
                    inst.sync_info = mybir.SyncInfo(
                        on_wait=[waits[-1]], on_update=list(si.on_update))
                out.append(inst)
            blk.instructions = out
    return cnt


def build_program():
    _patch_sem_clear()
    nc = bass.Bass("TRN2")
    xin = nc.dram_tensor("xin", [NP, 4 * PP], F16, kind="ExternalInput")
    vin = nc.dram_tensor("vin", [NP, NPIX], F16, kind="ExternalInput")
    ain = nc.dram_tensor("ain", [NP, 36 * NPIX], F16, kind="ExternalInput")
    ooutA = nc.dram_tensor("ooutA", [NP, 2 * NPIX], F32, kind="ExternalOutput")
    ooutB = nc.dram_tensor("ooutB", [NP, 2 * NPIX], F32, kind="ExternalOutput")

    with tile.TileContext(nc) as tc, \
         nc.allow_low_precision(reason="fp16 main path; fp32 psum accum"):
        with tc.tile_pool(name="persist", bufs=1) as pp, \
             tc.tile_pool(name="work", bufs=6) as wp, \
             tc.tile_pool(name="psum", bufs=1, space="PSUM") as qp:
            X = pp.tile([NP, 4 * PP], F16, tag="X")
            v = pp.tile([NP, NPIX], F16, tag="v")
            A = pp.tile([NP, 36 * NPIX], F16, tag="A")
            ident = pp.tile([128, 128], F16, tag="ident")
            obA = pp.tile([NP, 2 * NPIX], F32, tag="obA")
            obB = pp.tile([NP, 2 * NPIX], F32, tag="obB")

            # split X across both HWDGE queues to halve the startup stall
            nc.sync.dma_start(X[:, 0:2 * PP], xin[:, 0:2 * PP])
            nc.scalar.dma_start(X[:, 2 * PP:4 * PP], xin[:, 2 * PP:4 * PP])
            nc.sync.dma_start(v[:, :], vin[:, :])
            nc.sync.dma_start(A[:, :], ain[:, :])
            make_identity(nc, ident[:, :])

            psumA = qp.tile([128, 512], F32, tag="psA")  # planes x0,x1
            psumB = qp.tile([128, 512], F32, tag="psB")  # planes x2,den

            xap = X[:, :]
            CENTER = PAD * PT + PAD

            # center term: psum <- [x0,x1] , [x2,1] (weight exactly 1).
            # NOTE: start=True resets the whole PSUM bank -- exactly one
            # start per psum tile.
            cA = _sub(xap, [[PP, 2], [PT, T], [1, T]], CENTER)
            cB = _sub(xap, [[PP, 2], [PT, T], [1, T]], 2 * PP + CENTER)
            nc.tensor.matmul(psumA[:, :], ident[:, :], cA,
                             start=True, stop=False)
            mm = nc.tensor.matmul(psumB[:, :], ident[:, :], cB,
                                  start=True, stop=False)
            mm.ldweights = False  # identity stays resident in the PE array

            for pi, pair in enumerate(PAIRS):
                npr = len(pair)
                last_pair = pi == len(PAIRS) - 1
                # D = [rawA3 | rawB3 | sqA3 | sqB3] planes at stride MAXNEL
                D = wp.tile([NP, 12 * MAXNEL], F16, tag="D")
                E = wp.tile([NP, 2 * MAXNEL], F16, tag="E")
                t2p = wp.tile([NP, 1024], F16, tag="t2")
                s2p = wp.tile([NP, 1024], F16, tag="s2")
                C = wp.tile([NP, 4096], F16, tag="C")

                # pseudo-pairs (unequal i) pad to the wider window; the
                # extra columns read in-bounds halo junk that is never used
                j0 = TAPS[pair[0]][0]
                nr = T + abs(j0)
                ncol = T + max(TAPS[t][1] for t in pair)
                nel = nr * ncol

                for s, ti in enumerate(pair):
                    j, i, d2, m = TAPS[ti]
                    rlo_s = min(0, -j)
                    w0 = (PAD + rlo_s) * PT + (PAD - i)
                    w1 = (PAD + rlo_s + j) * PT + PAD
                    in0 = _sub(xap, [[PP, 3], [PT, nr], [1, ncol]], w0)
                    in1 = _sub(xap, [[PP, 3], [PT, nr], [1, ncol]], w1)
                    dap = _sub(D[:, :], [[MAXNEL, 3], [1, nel]],
                               s * 3 * MAXNEL)
                    nc.vector.tensor_tensor(out=dap, in0=in0, in1=in1,
                                            op=ALU.subtract)

                # one ACT square for the whole pair
                dln = _sub(D[:, :], [[1, npr * 3 * MAXNEL]], 0)
                dsq = _sub(D[:, :], [[1, npr * 3 * MAXNEL]], 6 * MAXNEL)
                nc.scalar.activation(dsq, dln, ACTF.Square)

                # E_s = sq0 + sq1 + sq2 (both taps in one op)
                e = _sub(E[:, :], [[MAXNEL, npr], [1, nel]], 0)
                nc.vector.tensor_tensor(
                    out=e,
                    in0=_sub(D[:, :], [[3 * MAXNEL, npr], [1, nel]],
                             6 * MAXNEL),
                    in1=_sub(D[:, :], [[3 * MAXNEL, npr], [1, nel]],
                             7 * MAXNEL), op=ALU.add)
                nc.vector.tensor_tensor(
                    out=e, in0=e,
                    in1=_sub(D[:, :], [[3 * MAXNEL, npr], [1, nel]],
                             8 * MAXNEL), op=ALU.add)

                slot_dirs = []
                for s, ti in enumerate(pair):
                    j, i, d2, m = TAPS[ti]
                    rlo_s = min(0, -j)
                    # E(q) = ||x(q) - x(q+d)||^2 for q in the extended window
                    offd = ((0 - rlo_s) * ncol + i,
                            (-j - rlo_s) * ncol + 0)
                    base = min(offd) + s * MAXNEL
                    step = abs(offd[1] - offd[0])
                    slot_dir = (0, 1) if offd[0] <= offd[1] else (1, 0)
                    slot_dirs.append(slot_dir)

                    e2 = _sub(E[:, :], [[step, 2], [ncol, T], [1, T]], base)
                    v2 = _sub(v[:, :], [[0, 2], [1, NPIX]], 0)
                    t2a = _sub(t2p[:, :], [[1, 512]], s * 512)
                    nc.vector.tensor_tensor(out=t2a, in0=e2, in1=v2,
                                            op=ALU.mult)

                # s2 = t2 + A, both taps in one op (taps adjacent in A)
                nc.vector.tensor_tensor(
                    out=_sub(s2p[:, :], [[1, npr * 512]], 0),
                    in0=_sub(t2p[:, :], [[1, npr * 512]], 0),
                    in1=_sub(A[:, :], [[NPIX, npr], [0, 2], [1, NPIX]],
                             pair[0] * NPIX),
                    op=ALU.add)

                # one exp for the whole pair -> C's w slots
                sin = _sub(s2p[:, :], [[1, npr * 512]], 0)
                wap = _sub(C[:, :], [[1024, 2 * npr], [1, NPIX]], 768)
                nc.scalar.activation(wap, sin, ACTF.Exp, scale=-1.0)

                # prod3 = w * [x0,x1,x2](shifted) per tap-slot and direction
                for s, ti in enumerate(pair):
                    j, i, d2, m = TAPS[ti]
                    for slot in range(2):
                        d = slot_dirs[s][slot]
                        sgn = 1 if d == 0 else -1
                        co = s * 2048 + slot * 1024
                        wbr = _sub(C[:, :], [[0, 3], [1, NPIX]], co + 768)
                        xw = _sub(xap, [[PP, 3], [PT, T], [1, T]],
                                  (PAD + sgn * j) * PT + (PAD + sgn * i))
                        pr = _sub(C[:, :], [[1, 768]], co)
                        nc.vector.tensor_tensor(out=pr, in0=wbr, in1=xw,
                                                op=ALU.mult)

                # psum accumulation on PE: A += [p0,p1], B += [p2,w]
                for s in range(npr):
                    for slot in range(2):
                        stop = last_pair and s == npr - 1 and slot == 1
                        co = s * 2048 + slot * 1024
                        rA = _sub(C[:, :], [[1, 512]], co)
                        rB = _sub(C[:, :], [[1, 512]], co + 512)
                        mm = nc.tensor.matmul(psumA[:, :], ident[:, :], rA,
                                              start=False, stop=stop)
                        mm.ldweights = False
                        mm = nc.tensor.matmul(psumB[:, :], ident[:, :], rB,
                                              start=False, stop=stop)
                        mm.ldweights = False

            # ship raw [num0,num1,num2,den] planes; host divides + rescales.
            # Two independent copy+DMA chains (DVE+sync vs ACT+scalar) so the
            # tail halves; PSUM cannot DMA directly.
            nc.vector.tensor_copy(out=obA[:, :], in_=psumA[:, :])
            nc.sync.dma_start(ooutA[:, :], obA[:, :])
            nc.scalar.activation(obB[:, :], psumB[:, :], ACTF.Copy)
            nc.scalar.dma_start(ooutB[:, :], obB[:, :])
    _legalize_waits(nc)
    return nc


def host_shard(x, sigmaD, sigmaR):
    """x [1,3,512,512] -> per-core inputs. Pure gather/pad/scale prep."""
    from numpy.lib.stride_tricks import sliding_window_view
    xs = x[0] * (np.array(SCALE, np.float32) / 4.0)[:, None, None]
    xg = np.pad(xs, ((0, 0), (PAD, PAD), (PAD, PAD)), mode="edge")
    swv = sliding_window_view(xg, (PT, PT), axis=(1, 2))
    blocks = swv[:, ::T, ::T][:, :32, :32]                # [3, 32, 32, 24, 24]
    ones = np.ones((1,) + blocks.shape[1:], np.float32)
    x4 = np.concatenate([blocks, ones], axis=0)           # [4, 32, 32, 24, 24]
    tiles = np.ascontiguousarray(
        x4.transpose(1, 2, 0, 3, 4)).astype(np.float16)   # [32,32,4,24,24]
    tiles = tiles.reshape(NCORES, NP, 4 * PP)

    sd, sr = sigmaD[0, 0], sigmaR[0, 0]
    u = 1.0 / (0.5 * sd * sd + EPS)
    v16 = 16.0 / (2.0 * sr * sr + EPS)
    # A_k = d2_k*u + 100*(tap k inactive)
    A = np.stack([d2 * u + 100.0 * (sd <= float(m - 1))
                  for (_, _, d2, m) in TAPS])

    def tile_sig(s):  # [k?,512,512] -> [NCORES, NP, k?*256] tile-major
        k = s.shape[0] if s.ndim == 3 else 1
        s = s.reshape(k, 32, T, 32, T).transpose(1, 3, 0, 2, 4)
        return np.ascontiguousarray(s).reshape(NCORES, NP, k * NPIX)

    vt = tile_sig(v16[None]).astype(np.float16)
    at = tile_sig(A).astype(np.float16)
    return [{"xin": tiles[c], "vin": vt[c], "ain": at[c]}
            for c in range(NCORES)]


def assemble(results):
    rescale = (4.0 / np.array(SCALE, np.float32))[:, None, None]
    out = np.empty((1, 3, H, W), np.float32)
    for c, r in enumerate(results):
        o = np.concatenate([r["ooutA"], r["ooutB"]], axis=1)
        o = o.reshape(4, TRC, 4, T, T)
        # [tr, tc, plane, r, cc] -> [plane, tr, r, tc, cc]
        o = o.transpose(2, 0, 3, 1, 4).reshape(4, 64, W)
        out[0, :, c * 64:(c + 1) * 64, :] = o[:3] * rescale / o[3]
    return out


_NC_CACHE = {}


def get_nc():
    if "nc" not in _NC_CACHE:
        _NC_CACHE["nc"] = build_program()
    return _NC_CACHE["nc"]


def kernel(x, sigmaD, sigmaR, trace=False):
    x = np.asarray(x, np.float32)
    sigmaD = np.asarray(sigmaD, np.float32)
    sigmaR = np.asarray(sigmaR, np.float32)
    in_maps = host_shard(x, sigmaD, sigmaR)
    nc = get_nc()
    res = run_bass_kernel_spmd(nc, in_maps, list(range(NCORES)), trace=trace)
    out = assemble(res.results)
    kernel.last_result = res
    return out


# revision 51
# speedup vs baseline: 1.0119x; 1.0060x over previous
"""Bilateral effect kernel for Trainium2 (8 NeuronCores, SPMD).

Algorithm (matches reference.py):
  For each pixel p and tap delta=(j,i), j in [-4,4], i in [1,4] (taps with
  max(i,|j|)=5 are never active since sigmaD<4):
    w(p,+d) = exp(-(E(p,p+d)*v(p) + A_k(p)))
    w(p,-d) = exp(-(E(p-d,p)*v(p) + A_k(p)))
    A_k     = d2*u + 100*(tap k inactive),  u = 1/(0.5*sigmaD^2+eps)
    E(a,b)  = sum_c scale_c^2 (x_c[a]-x_c[b])^2,  scale=(100,254,254)
    v = 1/(2*sigmaR^2+eps)
    out_c = (x_c + sum w*x_c[shifted]) / (1 + sum w)
  (exp(-100) underflows to exactly 0 in fp16 -> the inactive-tap mask folded
   into the host-precomputed A planes is exact.)

Layout: every NeuronCore gets 64 image rows = 128 sub-tiles of 16x16 center
pixels; each SBUF partition owns one sub-tile padded to 24x24, stored as 4
fp16 planes [x0,x1,x2,ones] (halo+edge replication+scaling done host-side).
All taps are pure free-dim shifted reads.

Engine split (taps processed as (j,i)/(-j,i) pairs):
  DVE : per tap a planar 3-ch sub, E = channel-sum (2 pair-merged adds),
        E*v (2 dirs packed per op), +A add, 2 broadcast prod mults.
  ACT : one Square per pair (both taps' diffs), one exp per pair writing
        all 4 dirs' w straight into the combined prod/w buffer.
  PE  : psum += I @ [prod3|w] (512-col fp16 matmuls) -- numerator AND
        denominator accumulate on the tensor engine; A = d2*u + mask and
        v = 16/(2*sigmaR^2+eps) come precomputed from the host.
"""
import dataclasses
import numpy as np

import concourse.bass as bass
import concourse.mybir as mybir
import concourse.tile as tile
from concourse.bass_utils import run_bass_kernel_spmd
from concourse.masks import make_identity

F32 = mybir.dt.float32
F16 = mybir.dt.float16
ALU = mybir.AluOpType
ACTF = mybir.ActivationFunctionType

H = W = 512
NCORES = 8
T = 16            # center tile side
PAD = 4           # halo
PT = T + 2 * PAD  # 24 padded tile side
NP = 128          # partitions (tiles) per core
TRC = 32          # tile-cols per core (512/16); tile-rows per core = 4
EPS = float(np.finfo(np.float32).eps)
SCALE = (100.0, 254.0, 254.0)
NPIX = T * T      # 256
PP = PT * PT      # 576 plane size
MAXNEL = 20 * 20  # max extended-window size

# live taps: (j=row off, i=col off, d2, m), ordered so that each group is
# processed together (one ACT square / one exp per group). (j,i)/(-j,i) are
# natural pairs; the j=0 taps are pseudo-paired (padded to the wider window).
_PAIR_KEYS = [[(j, i), (-j, i)] for i in range(1, 5) for j in range(1, 5)]
_PAIR_KEYS += [[(0, 1), (0, 2)], [(0, 3), (0, 4)]]
TAPS = [(j, i, float(i * i + j * j), max(i, abs(j)))
        for grp in _PAIR_KEYS for (j, i) in grp]
assert len(TAPS) == 36
_k = iter(range(36))
PAIRS = [[next(_k) for _ in grp] for grp in _PAIR_KEYS]


def _sub(ap, dims, off):
    """AP over free dims of a pool tile: dims = [[step,count],...] (elements),
    off = element offset within the partition's free space."""
    return dataclasses.replace(
        ap, ap=[list(ap.ap[0])] + [[int(s), int(c)] for s, c in dims],
        offset=int(off))


def _patch_sem_clear():
    """The walrus build in this container rejects the
    EVENT_SEMAPHORE_RANGE_CLEAR InstISA that Tile's kernel-tail drain emits
    ("ISA wrong length").  Replace it with per-semaphore nops carrying
    sem-wr-imm(0) updates, keeping the original free-list bookkeeping."""
    if getattr(bass.Bass, "_semclear_patched", False):
        return
    from concourse.bass import SemaphoreHandle

    def clear_and_free_semaphores(self, sems):
        if not sems:
            return
        sem_nums = [s.num if isinstance(s, SemaphoreHandle) else s for s in sems]
        self.gpsimd.dma_reset(range(min(sem_nums), max(sem_nums) + 1))
        for n in sem_nums:
            inst = self.gpsimd.nop()
            inst.sync_info = mybir.SyncInfo(
                on_wait=[],
                on_update=[mybir.SyncUpdate(
                    sync_type="semaphore", id=int(n),
                    update_mode="sem-wr-imm", update_value=0)])
        self._state.prepend_free_semaphores(sem_nums)
        for poison_set in self._tile_sem_poison_stack:
            poison_set.update(sem_nums)

    bass.Bass.clear_and_free_semaphores = clear_and_free_semaphores
    bass.Bass._semclear_patched = True


# These either never carry inline waits or are sequencer-level (multi-wait ok).
_WAIT_EXEMPT = {
    "InstDMA", "InstDMACopy", "InstDmaTransposeAnt", "InstTensorLoad",
    "InstTensorSave", "InstEventSemaphore",
    "InstCall", "InstUnconditionalBranch", "InstISA", "InstRegisterMove",
}


def _legalize_waits(nc):
    """This container's walrus accepts at most ONE inline sync wait per
    compute instruction.  Split extras onto same-engine NoOps inserted just
    before the instruction (engine stalls at the nop first — semantics
    preserved)."""
    cnt = 0
    for f in nc.m.functions:
        for blk in f.blocks:
            out = []
            for inst in blk.instructions:
                si = inst.sync_info
                if (si is not None and len(si.on_wait) > 1
                        and type(inst).__name__ not in _WAIT_EXEMPT):
                    waits = list(si.on_wait)
                    for wextra in waits[:-1]:
                        nop = mybir.InstNoOp(
                            name=f"waitnop-{cnt}", engine=inst.engine)
                        cnt += 1
                        nop.sync_info = mybir.SyncInfo(
                            on_wait=[wextra], on_update=[])
                        out.append(nop)
                    inst.sync_info = mybir.SyncInfo(
                        on_wait=[waits[-1]], on_update=list(si.on_update))
                out.append(inst)
            blk.instructions = out
    return cnt


def build_program():
    _patch_sem_clear()
    nc = bass.Bass("TRN2")
    xin = nc.dram_tensor("xin", [NP, 3 * PP], F16, kind="ExternalInput")
    vin = nc.dram_tensor("vin", [NP, NPIX], F16, kind="ExternalInput")
    ain = nc.dram_tensor("ain", [NP, 36 * NPIX], F16, kind="ExternalInput")
    ooutA = nc.dram_tensor("ooutA", [NP, 2 * NPIX], F32, kind="ExternalOutput")
    ooutB = nc.dram_tensor("ooutB", [NP, 2 * NPIX], F32, kind="ExternalOutput")

    with tile.TileContext(nc) as tc, \
         nc.allow_low_precision(reason="fp16 main path; fp32 psum accum"):
        with tc.tile_pool(name="persist", bufs=1) as pp, \
             tc.tile_pool(name="work", bufs=6) as wp, \
             tc.tile_pool(name="psum", bufs=1, space="PSUM") as qp:
            X = pp.tile([NP, 3 * PP], F16, tag="X")
            v = pp.tile([NP, NPIX], F16, tag="v")
            A = pp.tile([NP, 36 * NPIX], F16, tag="A")
            ident = pp.tile([128, 128], F16, tag="ident")
            obA = pp.tile([NP, 2 * NPIX], F32, tag="obA")
            obB = pp.tile([NP, 2 * NPIX], F32, tag="obB")

            # split X across both HWDGE queues to halve the startup stall
            nc.sync.dma_start(X[:, 0:2 * PP], xin[:, 0:2 * PP])
            nc.scalar.dma_start(X[:, 2 * PP:3 * PP], xin[:, 2 * PP:3 * PP])
            nc.sync.dma_start(v[:, :], vin[:, :])
            nc.sync.dma_start(A[:, :], ain[:, :])
            make_identity(nc, ident[:, :])

            psumA = qp.tile([128, 512], F32, tag="psA")  # planes x0,x1
            psumB = qp.tile([128, 512], F32, tag="psB")  # planes x2,den

            xap = X[:, :]
            CENTER = PAD * PT + PAD

            # psumA center term [x0,x1] (weight exactly 1). NOTE: start=True
            # resets the whole PSUM bank -- exactly one start per psum tile;
            # psumB's start rides on the very first tap B-matmul, the x2
            # center joins later as a start=False sub-range accumulate, and
            # den's +1 is applied host-side.
            cA = _sub(xap, [[PP, 2], [PT, T], [1, T]], CENTER)
            nc.tensor.matmul(psumA[:, :], ident[:, :], cA,
                             start=True, stop=False)

            for pi, pair in enumerate(PAIRS):
                npr = len(pair)
                last_pair = pi == len(PAIRS) - 1
                # D = [rawA3 | rawB3 | sqA3 | sqB3] planes at stride MAXNEL
                D = wp.tile([NP, 12 * MAXNEL], F16, tag="D")
                E = wp.tile([NP, 2 * MAXNEL], F16, tag="E")
                t2p = wp.tile([NP, 1024], F16, tag="t2")
                s2p = wp.tile([NP, 1024], F16, tag="s2")
                C = wp.tile([NP, 4096], F16, tag="C")

                # pseudo-pairs (unequal i) pad to the wider window; the
                # extra columns read in-bounds halo junk that is never used
                j0 = TAPS[pair[0]][0]
                nr = T + abs(j0)
                ncol = T + max(TAPS[t][1] for t in pair)
                nel = nr * ncol

                for s, ti in enumerate(pair):
                    j, i, d2, m = TAPS[ti]
                    rlo_s = min(0, -j)
                    w0 = (PAD + rlo_s) * PT + (PAD - i)
                    w1 = (PAD + rlo_s + j) * PT + PAD
                    in0 = _sub(xap, [[PP, 3], [PT, nr], [1, ncol]], w0)
                    in1 = _sub(xap, [[PP, 3], [PT, nr], [1, ncol]], w1)
                    dap = _sub(D[:, :], [[MAXNEL, 3], [1, nel]],
                               s * 3 * MAXNEL)
                    nc.vector.tensor_tensor(out=dap, in0=in0, in1=in1,
                                            op=ALU.subtract)

                # one ACT square for the whole pair
                dln = _sub(D[:, :], [[1, npr * 3 * MAXNEL]], 0)
                dsq = _sub(D[:, :], [[1, npr * 3 * MAXNEL]], 6 * MAXNEL)
                nc.scalar.activation(dsq, dln, ACTF.Square)

                # E_s = sq0 + sq1 + sq2 (both taps in one op)
                e = _sub(E[:, :], [[MAXNEL, npr], [1, nel]], 0)
                nc.vector.tensor_tensor(
                    out=e,
                    in0=_sub(D[:, :], [[3 * MAXNEL, npr], [1, nel]],
                             6 * MAXNEL),
                    in1=_sub(D[:, :], [[3 * MAXNEL, npr], [1, nel]],
                             7 * MAXNEL), op=ALU.add)
                nc.vector.tensor_tensor(
                    out=e, in0=e,
                    in1=_sub(D[:, :], [[3 * MAXNEL, npr], [1, nel]],
                             8 * MAXNEL), op=ALU.add)

                slot_dirs = []
                for s, ti in enumerate(pair):
                    j, i, d2, m = TAPS[ti]
                    rlo_s = min(0, -j)
                    # E(q) = ||x(q) - x(q+d)||^2 for q in the extended window
                    offd = ((0 - rlo_s) * ncol + i,
                            (-j - rlo_s) * ncol + 0)
                    base = min(offd) + s * MAXNEL
                    step = abs(offd[1] - offd[0])
                    slot_dir = (0, 1) if offd[0] <= offd[1] else (1, 0)
                    slot_dirs.append(slot_dir)

                    e2 = _sub(E[:, :], [[step, 2], [ncol, T], [1, T]], base)
                    v2 = _sub(v[:, :], [[0, 2], [1, NPIX]], 0)
                    t2a = _sub(t2p[:, :], [[1, 512]], s * 512)
                    nc.vector.tensor_tensor(out=t2a, in0=e2, in1=v2,
                                            op=ALU.mult)

                # s2 = t2 + A, both taps in one op (taps adjacent in A)
                nc.vector.tensor_tensor(
                    out=_sub(s2p[:, :], [[1, npr * 512]], 0),
                    in0=_sub(t2p[:, :], [[1, npr * 512]], 0),
                    in1=_sub(A[:, :], [[NPIX, npr], [0, 2], [1, NPIX]],
                             pair[0] * NPIX),
                    op=ALU.add)

                # one exp for the whole pair -> C's w slots
                sin = _sub(s2p[:, :], [[1, npr * 512]], 0)
                wap = _sub(C[:, :], [[1024, 2 * npr], [1, NPIX]], 768)
                nc.scalar.activation(wap, sin, ACTF.Exp, scale=-1.0)

                # prod3 = w * [x0,x1,x2](shifted) per tap-slot and direction
                for s, ti in enumerate(pair):
                    j, i, d2, m = TAPS[ti]
                    for slot in range(2):
                        d = slot_dirs[s][slot]
                        sgn = 1 if d == 0 else -1
                        co = s * 2048 + slot * 1024
                        wbr = _sub(C[:, :], [[0, 3], [1, NPIX]], co + 768)
                        xw = _sub(xap, [[PP, 3], [PT, T], [1, T]],
                                  (PAD + sgn * j) * PT + (PAD + sgn * i))
                        pr = _sub(C[:, :], [[1, 768]], co)
                        nc.vector.tensor_tensor(out=pr, in0=wbr, in1=xw,
                                                op=ALU.mult)

                # psum accumulation on PE: A += [p0,p1], B += [p2,w]
                for s in range(npr):
                    for slot in range(2):
                        stop = last_pair and s == npr - 1 and slot == 1
                        firstB = pi == 0 and s == 0 and slot == 0
                        co = s * 2048 + slot * 1024
                        rA = _sub(C[:, :], [[1, 512]], co)
                        rB = _sub(C[:, :], [[1, 512]], co + 512)
                        mm = nc.tensor.matmul(psumA[:, :], ident[:, :], rA,
                                              start=False, stop=stop)
                        mm.ldweights = False
                        mm = nc.tensor.matmul(psumB[:, :], ident[:, :], rB,
                                              start=firstB, stop=stop)
                        mm.ldweights = False
                if pi == 0:
                    # x2 center joins psumB after its bank got started
                    cB = _sub(xap, [[PP, 1], [PT, T], [1, T]],
                              2 * PP + CENTER)
                    mm = nc.tensor.matmul(psumB[:, 0:256], ident[:, :], cB,
                                          start=False, stop=False)
                    mm.ldweights = False

            # ship raw [num0,num1,num2,den] planes; host divides + rescales.
            # Two independent copy+DMA chains (DVE+sync vs ACT+scalar) so the
            # tail halves; PSUM cannot DMA directly.
            nc.vector.tensor_copy(out=obA[:, :], in_=psumA[:, :])
            nc.sync.dma_start(ooutA[:, :], obA[:, :])
            nc.scalar.activation(obB[:, :], psumB[:, :], ACTF.Copy)
            nc.scalar.dma_start(ooutB[:, :], obB[:, :])
    _legalize_waits(nc)
    return nc


def host_shard(x, sigmaD, sigmaR):
    """x [1,3,512,512] -> per-core inputs. Pure gather/pad/scale prep."""
    from numpy.lib.stride_tricks import sliding_window_view
    xs = x[0] * (np.array(SCALE, np.float32) / 4.0)[:, None, None]
    xg = np.pad(xs, ((0, 0), (PAD, PAD), (PAD, PAD)), mode="edge")
    swv = sliding_window_view(xg, (PT, PT), axis=(1, 2))
    blocks = swv[:, ::T, ::T][:, :32, :32]                # [3, 32, 32, 24, 24]
    tiles = np.ascontiguousarray(
        blocks.transpose(1, 2, 0, 3, 4)).astype(np.float16)  # [32,32,3,24,24]
    tiles = tiles.reshape(NCORES, NP, 3 * PP)

    sd, sr = sigmaD[0, 0], sigmaR[0, 0]
    u = 1.0 / (0.5 * sd * sd + EPS)
    v16 = 16.0 / (2.0 * sr * sr + EPS)
    # A_k = d2_k*u + 100*(tap k inactive)
    A = np.stack([d2 * u + 100.0 * (sd <= float(m - 1))
                  for (_, _, d2, m) in TAPS])

    def tile_sig(s):  # [k?,512,512] -> [NCORES, NP, k?*256] tile-major
        k = s.shape[0] if s.ndim == 3 else 1
        s = s.reshape(k, 32, T, 32, T).transpose(1, 3, 0, 2, 4)
        return np.ascontiguousarray(s).reshape(NCORES, NP, k * NPIX)

    vt = tile_sig(v16[None]).astype(np.float16)
    at = tile_sig(A).astype(np.float16)
    return [{"xin": tiles[c], "vin": vt[c], "ain": at[c]}
            for c in range(NCORES)]


def assemble(results):
    rescale = (4.0 / np.array(SCALE, np.float32))[:, None, None]
    out = np.empty((1, 3, H, W), np.float32)
    for c, r in enumerate(results):
        o = np.concatenate([r["ooutA"], r["ooutB"]], axis=1)
        o = o.reshape(4, TRC, 4, T, T)
        # [tr, tc, plane, r, cc] -> [plane, tr, r, tc, cc]
        o = o.transpose(2, 0, 3, 1, 4).reshape(4, 64, W)
        out[0, :, c * 64:(c + 1) * 64, :] = o[:3] * rescale / (o[3] + 1.0)
    return out


_NC_CACHE = {}


def get_nc():
    if "nc" not in _NC_CACHE:
        _NC_CACHE["nc"] = build_program()
    return _NC_CACHE["nc"]


def kernel(x, sigmaD, sigmaR, trace=False):
    x = np.asarray(x, np.float32)
    sigmaD = np.asarray(sigmaD, np.float32)
    sigmaR = np.asarray(sigmaR, np.float32)
    in_maps = host_shard(x, sigmaD, sigmaR)
    nc = get_nc()
    res = run_bass_kernel_spmd(nc, in_maps, list(range(NCORES)), trace=trace)
    out = assemble(res.results)
    kernel.last_result = res
    return out
